# revision 1
# baseline (speedup 1.0000x reference)
"""Dendritic (per-block-softmax) attention kernel for Trainium2, 8 NeuronCores.

Math (per batch b, head h):
    qi     = q * importance[b, m]
    s[m,n] = (qi . k) / 8,  causal
    softmax per 64-wide key block (independent normalization per block):
        p = exp(s) / (sum_block exp(s) + 1e-6)          [masked entries -> 0]
    out[m] = sum_blocks p_block @ v_block
    out[m] = 0 for 64-wide query blocks where no importance > 0.3

Sharding: B*H = 32 (b,h) pairs split 4-per-core across 8 cores (head/data
parallel, fully independent per core).

Device pipeline per (b,h) pair (all layouts chosen so no on-chip transposes
are ever needed):
  1. scores transposed  s_T[n,m] = kT_chunk.T @ qiT  (PE; D=64 so two
     128-key chunks are row-packed into the 128x128 array via tile_position)
  2. e_T = exp(s_T/8)  (ACT, PSUM->SBUF)
  3. causal mask: zero e_T where m < n (GPSIMD affine_select, diagonal
     chunks only)
  4. per-block denominators den[j,m] = blocksel.T @ e_T  (PE matmul with a
     0/1 block-selector as stationary operand, PSUM-accumulated; j = global
     64-key block index, stacked 32-rows-per-m-chunk in one PSUM bank via
     tile_position column groups)
  5. inv = 1/(den + 1e-6)  (DVE reciprocal_approx_accurate)
  6. inv broadcast back to key-row shape via a second selector matmul
     (PE, K=32), p_T = e_T * inv_bc  (DVE, the one full-area elementwise
     pass)
  7. PV: out[m,d] = p_T.T @ v  (PE, p_T stationary, natural-layout output)
  8. output exit fused with the importance-block gating: out_sb =
     out_psum * active01[m]  (DVE tensor_scalar with per-partition scalar)

The q/k transposed+packed layouts (contraction dim D on partitions), the
importance pre-scale of q, and v's chunk-tile layout are prepared host-side
in numpy as part of sharding.
"""

import os
import numpy as np
import ml_dtypes

# ---------------------------------------------------------------- constants
_B, _H, _S, _D = 2, 16, 2048, 64
_NCORES = 8
_PAIRS = (_B * _H) // _NCORES  # 4 (b,h) pairs per core
_BLK = 64                      # softmax key-block size (BLOCK_N)
_THR = 0.3                     # importance threshold (BLOCK_M gating)
_EPS = 1e-6
_SCALE = 0.125                 # 1/sqrt(D)
_MC = 512                      # m-chunk width (PSUM bank)
_NCK = 128                     # key chunk (partition dim)
_NMC = _S // _MC               # 4 m-chunks
_NNC = _S // _NCK              # 16 key chunks
_JBLK = _S // _BLK             # 32 key blocks
_NT = _S // 128                # 16 m-tiles of 128

# IO dtype for q/k/v/e/p on device. bf16: PE runs at full rate.
_USE_BF16 = bool(int(os.environ.get("KERNEL_BF16", "1")))
# Repeat the whole workload inside the device program (timing: the wall-time
# slope between REPS values isolates device time from dispatch overhead).
_REPS = int(os.environ.get("KERNEL_REPS", "1"))
# Dynamic-loop repeat count (tc.For_i around the whole body) for timing.
_LOOP = int(os.environ.get("KERNEL_LOOP", "1"))

_cache = {}


# ---------------------------------------------------------------- device IR
def _build_program(loop=None):
    import concourse.bass as bass
    import concourse.tile as tile
    from concourse import bacc, mybir
    from contextlib import ExitStack

    loop = _LOOP if loop is None else loop

    f32 = mybir.dt.float32
    dio = mybir.dt.bfloat16 if _USE_BF16 else f32
    EXP = mybir.ActivationFunctionType.Exp
    OP = mybir.AluOpType

    nc = bacc.Bacc(
        "TRN2", target_bir_lowering=False, debug=False, num_devices=_NCORES
    )
    qiT_d = nc.dram_tensor("qiT", [_PAIRS, 128, _S], dio, kind="ExternalInput").ap()
    kTp_d = nc.dram_tensor("kTp", [_PAIRS, 128, _S // 2], dio, kind="ExternalInput").ap()
    vt_d = nc.dram_tensor("vt", [_PAIRS, 128, _NNC * _D], dio, kind="ExternalInput").ap()
    imp_d = nc.dram_tensor("impn", [_PAIRS, 128, _NT], f32, kind="ExternalInput").ap()
    out_d = nc.dram_tensor("out", [_PAIRS, _S, _D], f32, kind="ExternalOutput").ap()

    with tile.TileContext(nc) as tc, ExitStack() as ctx:
        cpool = ctx.enter_context(tc.tile_pool(name="consts", bufs=1))
        inpool = ctx.enter_context(tc.tile_pool(name="inputs", bufs=2))
        epool = ctx.enter_context(
            tc.tile_pool(
                name="etiles", bufs=int(os.environ.get("KERNEL_EBUFS", "42"))
            )
        )
        spool = ctx.enter_context(tc.tile_pool(name="small", bufs=4))
        opool = ctx.enter_context(tc.tile_pool(name="outsb", bufs=3))
        # PSUM budget: 8 banks total; [128,1024] tiles take 2 banks each.
        _g = lambda k, d: int(os.environ.get(k, str(d)))
        split_exp = bool(_g("KERNEL_SPLITEXP", 1))
        psA = ctx.enter_context(
            tc.tile_pool(name="psA", bufs=_g("KERNEL_PSA", 2), space="PSUM")
        )
        psDen = ctx.enter_context(
            tc.tile_pool(name="psDen", bufs=_g("KERNEL_PSDEN", 1), space="PSUM")
        )
        psBc = ctx.enter_context(
            tc.tile_pool(name="psBc", bufs=_g("KERNEL_PSBC", 2), space="PSUM")
        )
        psOut = ctx.enter_context(
            tc.tile_pool(name="psOut", bufs=_g("KERNEL_PSOUT", 1), space="PSUM")
        )

        # ---------------- selector constants (built once, on GPSIMD) -----
        # selG[p, c, j] = 1 iff j == 2c + p//64   (den matmul stationary)
        selG = cpool.tile([128, _NNC, 32], dio, tag="selG")
        nc.gpsimd.memset(selG[:], 1.0)
        nc.gpsimd.affine_select(
            out=selG[0:64], in_=selG[0:64], compare_op=OP.is_equal, fill=0.0,
            base=0, channel_multiplier=0, pattern=[[-2, _NNC], [1, 32]],
        )
        nc.gpsimd.affine_select(
            out=selG[64:128], in_=selG[64:128], compare_op=OP.is_equal, fill=0.0,
            base=-1, channel_multiplier=0, pattern=[[-2, _NNC], [1, 32]],
        )

        # selRE[j, c, n] = 1 iff j == 2c + n//64  (inv broadcast stationary)
        # built on partitions 0..31, then replicated to the other 3 row
        # groups with SBUF->SBUF DMAs (so matmul rhs/lhsT partition bases
        # line up for every m-chunk's row group).
        selRE = cpool.tile([128, _NNC, 128], dio, tag="selRE")
        nc.gpsimd.memset(selRE[0:32], 1.0)
        nc.gpsimd.affine_select(
            out=selRE[0:32], in_=selRE[0:32], compare_op=OP.is_ge, fill=0.0,
            base=0, channel_multiplier=-64, pattern=[[128, _NNC], [1, 128]],
        )
        nc.gpsimd.affine_select(
            out=selRE[0:32], in_=selRE[0:32], compare_op=OP.is_ge, fill=0.0,
            base=63, channel_multiplier=64, pattern=[[-128, _NNC], [-1, 128]],
        )
        for g in range(1, 4):
            nc.sync.dma_start(selRE[32 * g : 32 * g + 32], selRE[0:32])

        # blksel2[p, j] = 1 iff j == p//64  (importance-block counts)
        blksel2 = cpool.tile([128, 2], f32, tag="blksel2")
        nc.gpsimd.memset(blksel2[:], 1.0)
        nc.gpsimd.affine_select(
            out=blksel2[0:64], in_=blksel2[0:64], compare_op=OP.is_equal,
            fill=0.0, base=0, channel_multiplier=0, pattern=[[1, 2]],
        )
        nc.gpsimd.affine_select(
            out=blksel2[64:128], in_=blksel2[64:128], compare_op=OP.is_equal,
            fill=0.0, base=-1, channel_multiplier=0, pattern=[[1, 2]],
        )

        # eps bias vector for the denominator (ACT bias must be an AP)
        eps_t = cpool.tile([128, 1], f32, tag="eps_t")
        nc.vector.memset(eps_t[:], _EPS)

        # selT[j, p] = 1 iff p//64 == j  (count -> per-partition broadcast)
        selT = cpool.tile([2, 128], f32, tag="selT")
        nc.gpsimd.memset(selT[:], 1.0)
        nc.gpsimd.affine_select(
            out=selT[:], in_=selT[:], compare_op=OP.is_ge, fill=0.0,
            base=0, channel_multiplier=-64, pattern=[[1, 128]],
        )
        nc.gpsimd.affine_select(
            out=selT[:], in_=selT[:], compare_op=OP.is_ge, fill=0.0,
            base=63, channel_multiplier=64, pattern=[[-1, 128]],
        )

        # ---------------- per (b,h) pair ---------------------------------
        loop_cm = tc.For_i(0, loop, 1) if loop > 1 else None
        if loop_cm is not None:
            loop_cm.__enter__()
        for pp in [p for _ in range(_REPS) for p in range(_PAIRS)]:
            qiT = inpool.tile([128, _S], dio, tag="qiT")
            nc.sync.dma_start(qiT[:], qiT_d[pp])
            kTp = inpool.tile([128, _S // 2], dio, tag="kTp")
            nc.sync.dma_start(kTp[:], kTp_d[pp])
            vt = inpool.tile([128, _NNC * _D], dio, tag="vt")
            nc.sync.dma_start(vt[:], vt_d[pp])
            impn = inpool.tile([128, _NT], f32, tag="impn")
            nc.sync.dma_start(impn[:], imp_d[pp])

            # ---- importance block gating -> act01[p, t] in {0.0, 1.0}
            ind = spool.tile([128, _NT], f32, tag="ind")
            nc.vector.tensor_scalar(ind[:], impn[:], _THR, None, OP.is_gt)
            cnt_ps = psOut.tile([2, _NT], f32, tag="o_ps")
            nc.tensor.matmul(cnt_ps[:], blksel2[:], ind[:], start=True, stop=True)
            cnt = spool.tile([2, _NT], f32, tag="cnt")
            nc.vector.tensor_copy(cnt[:], cnt_ps[:])
            act_ps = psOut.tile([128, _NT], f32, tag="o_ps")
            nc.tensor.matmul(act_ps[:], selT[:], cnt[:], start=True, stop=True)
            act01 = spool.tile([128, _NT], f32, tag="act01")
            nc.vector.tensor_scalar(act01[:], act_ps[:], 0.5, None, OP.is_gt)

            # mc-groups: each group runs scores->exp->mask->den, then its own
            # reciprocal, broadcast+normalize, and PV — so group g+1's phase 1
            # overlaps group g's tail (and pairs overlap at both ends).
            ngroups = _g("KERNEL_GROUPS", 1)
            mcs_per = _NMC // ngroups
            etiles = {}
            for grp in range(ngroups):
                mclist = range(grp * mcs_per, (grp + 1) * mcs_per)
                rl, rh = 32 * mclist.start, 32 * mclist.stop
                den_ps = psDen.tile([128, _MC], f32, tag="den")

                # -- phase 1: scores -> exp -> mask -> den accumulation ---
                for mc in mclist:
                    ncnt = 4 * (mc + 1)  # causal: chunks 0 .. 4*mc+3
                    for nc_i in range(0, ncnt, 2):
                        cpair = nc_i // 2
                        # fully-masked column window per half: cols < lo(nci)
                        # are never read downstream (PV's m-tile skip implies
                        # it), so scores/exp/mask/den all shrink to [lo:512].
                        los = [
                            max(0, _NCK * (nc_i + w) - _MC * mc) for w in (0, 1)
                        ]
                        et = epool.tile([128, 2 * _MC], dio, tag="eT")
                        if split_exp:
                            halves = [
                                psA.tile([128, _MC], f32, tag="sT", name="sT_a"),
                                psA.tile([128, _MC], f32, tag="sT", name="sT_b"),
                            ]
                            s_slices = [
                                h[:, los[w] : _MC] for w, h in enumerate(halves)
                            ]
                        else:
                            s01 = psA.tile([128, 2 * _MC], f32, tag="sT")
                            s_slices = [
                                s01[:, los[0] : _MC],
                                s01[:, _MC + los[1] : 2 * _MC],
                            ]
                        for w in (0, 1):
                            nc.tensor.matmul(
                                s_slices[w],
                                kTp[64 * w : 64 * w + 64,
                                    128 * cpair : 128 * cpair + 128],
                                qiT[64 * w : 64 * w + 64,
                                    _MC * mc + los[w] : _MC * (mc + 1)],
                                start=True, stop=True, tile_position=(64 * w, 0),
                            )
                        if not split_exp and los == [0, 0]:
                            nc.scalar.activation(et[:], s01[:], EXP, scale=_SCALE)
                        else:
                            for w in (0, 1):
                                nc.scalar.activation(
                                    et[:, _MC * w + los[w] : _MC * (w + 1)],
                                    s_slices[w], EXP, scale=_SCALE,
                                )
                        if nc_i + 1 >= 4 * mc:
                            # diagonal pair-tile: zero both halves where
                            # m < n with one 2D affine_select (also cleans
                            # the never-written skip windows to 0)
                            nc.gpsimd.affine_select(
                                out=et[:].rearrange("p (w m) -> p w m", m=_MC),
                                in_=et[:].rearrange("p (w m) -> p w m", m=_MC),
                                compare_op=OP.is_ge, fill=0.0,
                                base=_MC * mc - _NCK * nc_i,
                                channel_multiplier=-1,
                                pattern=[[-_NCK, 2], [1, _MC]],
                            )
                        for which in (0, 1):
                            nci = nc_i + which
                            lo = los[which]
                            half = et[:, _MC * which + lo : _MC * (which + 1)]
                            etiles[(mc, nci)] = (et, _MC * which)
                            # den[j, m] accumulation into row group mc
                            nc.tensor.matmul(
                                den_ps[32 * mc : 32 * mc + 32, lo:_MC],
                                selG[:, nci, :],
                                half,
                                start=(nci == 0), stop=(nci == ncnt - 1),
                                tile_position=(0, 32 * mc),
                            )

                # -- phase 2: inv = 1/(den + eps) on this group's rows ----
                den_eps = spool.tile([128, _MC], f32, tag="den_eps")
                nc.scalar.activation(
                    den_eps[rl:rh], den_ps[rl:rh],
                    mybir.ActivationFunctionType.Identity, bias=eps_t[rl:rh],
                )
                inv_f = spool.tile([128, _MC], f32, tag="inv_f")
                scratch = spool.tile([128, _MC], f32, tag="scratch")
                nc.vector.reciprocal_approx_accurate(
                    inv_f[rl:rh], den_eps[rl:rh], scratch[rl:rh]
                )
                inv = spool.tile([128, _MC], dio, tag="inv")
                nc.vector.tensor_copy(inv[rl:rh], inv_f[rl:rh])

                # -- phase 3: broadcast inv, normalize e in place ---------
                for mc in mclist:
                    for nc_i in range(0, 4 * (mc + 1), 2):
                        los = [
                            max(0, _NCK * (nc_i + w) - _MC * mc) for w in (0, 1)
                        ]
                        bc = psBc.tile([128, 2 * _MC], f32, tag="bc")
                        for w in (0, 1):
                            nc.tensor.matmul(
                                bc[:, _MC * w + los[w] : _MC * (w + 1)],
                                selRE[32 * mc : 32 * mc + 32, nc_i + w, :],
                                inv[32 * mc : 32 * mc + 32, los[w] : _MC],
                                start=True, stop=True,
                                tile_position=(32 * mc, 0),
                            )
                        et, _ = etiles[(mc, nc_i)]
                        if los == [0, 0]:
                            nc.vector.tensor_tensor(et[:], et[:], bc[:], OP.mult)
                        else:
                            for w in (0, 1):
                                sl = slice(_MC * w + los[w], _MC * (w + 1))
                                nc.vector.tensor_tensor(
                                    et[:, sl], et[:, sl], bc[:, sl], OP.mult
                                )

                # -- phase 4: PV + gated exit + store ---------------------
                for mc in mclist:
                    o_ps = psOut.tile([128, 4 * _D], f32, tag="o_ps")
                    for t in range(4):
                        tt = 4 * mc + t
                        for nci in range(tt + 1):
                            et, cb = etiles[(mc, nci)]
                            nc.tensor.matmul(
                                o_ps[:, _D * t : _D * (t + 1)],
                                et[:, cb + 128 * t : cb + 128 * (t + 1)],
                                vt[:, _D * nci : _D * (nci + 1)],
                                start=(nci == 0), stop=(nci == tt),
                            )
                    o_sb = opool.tile([128, 4 * _D], f32, tag="o_sb")
                    for t in range(4):
                        # PSUM exit fused with the importance-block gating:
                        # out = Copy(o_ps * act01[m]) on the Scalar engine.
                        nc.scalar.activation(
                            o_sb[:, _D * t : _D * (t + 1)],
                            o_ps[:, _D * t : _D * (t + 1)],
                            mybir.ActivationFunctionType.Copy,
                            scale=act01[:, 4 * mc + t : 4 * mc + t + 1],
                        )
                    nc.sync.dma_start(
                        out_d[pp, _MC * mc : _MC * (mc + 1), :].rearrange(
                            "(t p) d -> p t d", p=128
                        ),
                        o_sb[:].rearrange("p (t d) -> p t d", d=_D),
                    )

        if loop_cm is not None:
            loop_cm.__exit__(None, None, None)

    nc.compile()
    return nc


# ---------------------------------------------------------------- host side
def _prep_inputs(q, k, v, importance_scores):
    """Shard + lay out the full inputs for the 8 cores.

    Core c gets flat (b,h) pairs [4c, 4c+4).  Layouts:
      qiT: importance-scaled q, transposed to [D, S], D replicated to 128
           partitions (for the row-packed score matmuls).
      kTp: k transposed to [D, S], packed [128, S/2]: rows 0:64 = even
           128-chunks, rows 64:128 = odd chunks.
      vt : v chunk tiles [128, 16*64]:  vt[p, 64*c+d] = v[128*c+p, d].
      impn: importance in natural m-tile layout [128, 16].
    """
    npdt = ml_dtypes.bfloat16 if _USE_BF16 else np.float32
    q = np.asarray(q, dtype=np.float32)
    k = np.asarray(k, dtype=np.float32)
    v = np.asarray(v, dtype=np.float32)
    imp = np.asarray(importance_scores, dtype=np.float32)

    qi = q * imp[:, None, :, None]                       # [B,H,S,D]
    qiT = np.ascontiguousarray(qi.transpose(0, 1, 3, 2))  # [B,H,D,S]
    qiT_rep = np.concatenate([qiT, qiT], axis=2)          # [B,H,128,S]

    kT = np.ascontiguousarray(k.transpose(0, 1, 3, 2))    # [B,H,D,S]
    # [B,H,D,16,128] -> even/odd chunk split -> [B,H,128,S/2]
    kc = kT.reshape(_B, _H, _D, _NNC, _NCK)
    kTp = np.concatenate(
        [kc[:, :, :, 0::2, :], kc[:, :, :, 1::2, :]], axis=2
    ).reshape(_B, _H, 128, _S // 2)

    vt = (
        v.reshape(_B, _H, _NNC, _NCK, _D)
        .transpose(0, 1, 3, 2, 4)
        .reshape(_B, _H, 128, _NNC * _D)
    )

    impn = (
        imp.reshape(_B, _NT, 128).transpose(0, 2, 1)      # [B,128,16]
    )

    in_maps = []
    for c in range(_NCORES):
        idx = [(f // _H, f % _H) for f in range(4 * c, 4 * c + 4)]
        in_maps.append(
            {
                "qiT": np.stack([qiT_rep[b, h] for b, h in idx]).astype(npdt),
                "kTp": np.stack([kTp[b, h] for b, h in idx]).astype(npdt),
                "vt": np.stack([vt[b, h] for b, h in idx]).astype(npdt),
                "impn": np.stack(
                    [impn[b] for b, h in idx]
                ).astype(np.float32),
            }
        )
    return in_maps


class _Runner:
    """Persistent jitted SPMD executor for a prebuilt Bass module.

    Mirrors concourse.bass2jax.run_bass_via_pjrt's multi-core path, but
    caches the jitted callable so repeated invocations don't re-trace,
    and exposes a device-resident call for timing.
    """

    def __init__(self, nc):
        import jax
        from jax.sharding import Mesh, PartitionSpec, NamedSharding
        from jax.experimental.shard_map import shard_map
        from concourse import mybir
        from concourse.bass2jax import (
            _bass_exec_p,
            install_neuronx_cc_hook,
            partition_id_tensor,
        )

        install_neuronx_cc_hook()
        assert nc.dbg_addr is None
        partition_name = (
            nc.partition_id_tensor.name if nc.partition_id_tensor else None
        )

        self.jax = jax
        in_names, out_names, out_avals = [], [], []
        for alloc in nc.m.functions[0].allocations:
            if not isinstance(alloc, mybir.MemoryLocationSet):
                continue
            name = alloc.memorylocations[0].name
            if alloc.kind == "ExternalInput":
                if name != partition_name:
                    in_names.append(name)
            elif alloc.kind == "ExternalOutput":
                out_names.append(name)
                out_avals.append(
                    jax.core.ShapedArray(
                        tuple(alloc.tensor_shape), mybir.dt.np(alloc.dtype)
                    )
                )
        self.in_names, self.out_names, self.out_avals = in_names, out_names, out_avals
        n_params, n_outs = len(in_names), len(out_avals)
        all_names = list(in_names + out_names)
        if partition_name is not None:
            all_names.append(partition_name)

        def _body(*args):
            operands = list(args)
            if partition_name is not None:
                operands.append(partition_id_tensor())
            outs = _bass_exec_p.bind(
                *operands,
                out_avals=tuple(out_avals),
                in_names=tuple(all_names),
                out_names=tuple(out_names),
                lowering_input_output_aliases=(),
                sim_require_finite=True,
                sim_require_nnan=True,
                nc=nc,
            )
            return tuple(outs)

        devices = jax.devices()[:_NCORES]
        assert len(devices) == _NCORES
        self.mesh = Mesh(np.asarray(devices), ("core",))
        self.sharding = NamedSharding(self.mesh, PartitionSpec("core"))
        donate = tuple(range(n_params, n_params + n_outs))
        self.fn = jax.jit(
            shard_map(
                _body,
                mesh=self.mesh,
                in_specs=(PartitionSpec("core"),) * (n_params + n_outs),
                out_specs=(PartitionSpec("core"),) * n_outs,
                check_rep=False,
            ),
            donate_argnums=donate,
            keep_unused=True,
        )

    def put_inputs(self, in_maps):
        cat = [
            np.concatenate([np.asarray(m[name]) for m in in_maps], axis=0)
            for name in self.in_names
        ]
        return [self.jax.device_put(a, self.sharding) for a in cat]

    def make_zero_outs(self):
        return [
            self.jax.device_put(
                np.zeros((_NCORES * av.shape[0], *av.shape[1:]), av.dtype),
                self.sharding,
            )
            for av in self.out_avals
        ]

    def __call__(self, dev_inputs, zero_outs):
        outs = self.fn(*dev_inputs, *zero_outs)
        return outs


def _get_runner(loop=None):
    loop = _LOOP if loop is None else loop
    key = ("runner", loop)
    if key not in _cache:
        _cache[key] = _Runner(_build_program(loop))
    return _cache[key]


def kernel(q, k, v, importance_scores):
    runner = _get_runner()
    in_maps = _prep_inputs(q, k, v, importance_scores)
    dev_in = runner.put_inputs(in_maps)
    _cache["bench_dev_in"] = dev_in
    outs = runner(dev_in, runner.make_zero_outs())
    out_cat = np.asarray(outs[0])  # [8*PAIRS, S, D]
    out = np.empty((_B, _H, _S, _D), dtype=np.float32)
    for f in range(_B * _H):
        out[f // _H, f % _H] = out_cat[f]
    return out


def bench(n_iters=20, loop=None):
    """Time repeated on-device executions (inputs resident, outputs donated).

    Returns (median, times) per-call wall seconds."""
    import time

    runner = _get_runner(loop)
    dev_in = _cache["bench_dev_in"]
    zsets = [runner.make_zero_outs() for _ in range(n_iters)]
    # warmup
    for o in runner(dev_in, runner.make_zero_outs()):
        o.block_until_ready()
    times = []
    for i in range(n_iters):
        t0 = time.perf_counter()
        outs = runner(dev_in, zsets[i])
        for o in outs:
            o.block_until_ready()
        times.append(time.perf_counter() - t0)
    return float(np.median(times)), times

